# revision 13
# baseline (speedup 1.0000x reference)
"""Trainium2 Bass kernel for nn_Attention_17454747091547.

Segmented-projection 2-head attention over seq=16, head_dim=3, batch 262144.
Pure data parallel across 8 NeuronCores (32768 batch elements per core).

Host-side precompute folds the per-position segment weights into 96x96
block-diagonal projection matrices, so QKV + out-proj become single TensorE
matmuls. x arrives host-pre-transposed as [96, 32768] bf16 and is preloaded
into SBUF in chunks; output is accumulated transposed in SBUF (bf16) and
written out with a few large DMAs. This keeps every PE/DMA instruction at
<=1 sync-wait (walrus codegen limit): all PE operands except the x chunks
are produced by DVE, and a setup dummy matmul makes PE observe the DVE
semaphore before tile 0.

The attention core (scores/softmax/attn@V) runs on VectorE/ScalarE with
batch on partitions.
"""

import numpy as np
import ml_dtypes

import concourse.bass as bass
import concourse.tile as tile
from concourse import bacc
from concourse import mybir
from concourse.bass_utils import run_bass_kernel_spmd

SEG = [0, 1, 1, 1, 1, 1, 2, 2, 2, 3, 4, 4, 4, 4, 4, 4]
N_CORES = 8
B_TOTAL = 262144
B_SHARD = B_TOTAL // N_CORES  # 32768
P = 128
NTILES = B_SHARD // P  # 256
TILES_PER_CHUNK = 8
NCHUNKS = NTILES // TILES_PER_CHUNK  # 32
CHUNK_COLS = TILES_PER_CHUNK * P  # 1024
S = 16
D = 6
H = 2
HD = 3
F = S * D  # 96

_nc_cache = {}


def _build_weight(W, scale=1.0):
    """[5,6,6] -> [96,96] f32 blockdiag of W[seg[s]].T."""
    Wa = np.zeros((F, F), np.float32)
    for s in range(S):
        Wa[s * D:(s + 1) * D, s * D:(s + 1) * D] = W[SEG[s]].T * scale
    return Wa


def _build_graph():
    nc = bacc.Bacc()
    f32 = mybir.dt.float32
    bf16 = mybir.dt.bfloat16

    xt_ext = nc.declare_dram_parameter("xt", [F, B_SHARD], bf16, isOutput=False)
    w_exts = {}
    b_exts = {}
    for nm in ["wq", "wk", "wv", "wo"]:
        w_exts[nm] = nc.declare_dram_parameter(nm, [F, F], bf16, isOutput=False)
    for nm in ["bq", "bk", "bv"]:
        b_exts[nm] = nc.declare_dram_parameter(nm, [F], bf16, isOutput=False)
    bo_ext = nc.declare_dram_parameter("bo", [F], f32, isOutput=False)
    id_ext = nc.declare_dram_parameter("ident", [P, P], bf16, isOutput=False)
    out_ext = nc.declare_dram_parameter("out", [F, B_SHARD], bf16, isOutput=True)

    mult = mybir.AluOpType.mult
    add = mybir.AluOpType.add

    with tile.TileContext(nc) as tc:
        with (
            tc.tile_pool(name="const", bufs=1) as const,
            tc.tile_pool(name="sbqkv", bufs=2) as sbqkv,
            tc.tile_pool(name="sbwork", bufs=2) as sbwork,
            tc.tile_pool(name="sbctxT", bufs=2) as sbctxT,
            tc.tile_pool(name="psT", bufs=2, space="PSUM") as psT,
            tc.tile_pool(name="psQKV", bufs=4, space="PSUM") as psQKV,
            tc.tile_pool(name="psO", bufs=2, space="PSUM") as psO,
        ):
            # --- setup: stage every PE-read constant through DVE ---
            w_dma = {}
            w_sb = {}
            for nm in ["wq", "wk", "wv", "wo"]:
                w_dma[nm] = const.tile([F, F], bf16, tag=nm + "d",
                                       name="wd_" + nm)
                nc.sync.dma_start(out=w_dma[nm], in_=w_exts[nm][:])
                w_sb[nm] = const.tile([F, F], bf16, tag=nm, name="w_" + nm)
                nc.vector.tensor_copy(w_sb[nm][:], w_dma[nm][:])
            id_dma = const.tile([P, P], bf16)
            nc.sync.dma_start(out=id_dma, in_=id_ext[:])
            I128b = const.tile([P, P], bf16)
            nc.vector.tensor_copy(I128b[:], id_dma[:])
            b_dma = {}
            b_sb = {}
            for nm in ["bq", "bk", "bv"]:
                b_dma[nm] = const.tile([P, F], bf16, tag=nm + "d",
                                       name="bd_" + nm)
                src = b_exts[nm][:]
                bcast = bass.AP(tensor=src.tensor, offset=src.offset,
                                ap=[[0, P]] + [list(d) for d in src.ap])
                nc.sync.dma_start(out=b_dma[nm], in_=bcast)
                b_sb[nm] = const.tile([P, F], bf16, tag=nm, name="b_" + nm)
                nc.vector.tensor_copy(b_sb[nm][:], b_dma[nm][:])
            bo_dma = const.tile([F, 1], f32)
            nc.sync.dma_start(out=bo_dma, in_=bo_ext[:].unsqueeze(1))
            bo_sb = const.tile([F, 1], f32)
            nc.vector.tensor_copy(bo_sb[:], bo_dma[:])

            # dummy matmul: PE observes the DVE setup tick, so tile-0 PE
            # instructions carry at most one sync-wait (walrus limit)
            psDummy = psT.tile([1, 1], f32, tag="t")
            nc.tensor.matmul(psDummy, lhsT=I128b[0:1, 0:1],
                             rhs=I128b[0:1, 0:1], start=True, stop=True)

            # x chunks: separate tiles so chunk DMAs have no mutual deps
            xc = []
            for c in range(NCHUNKS):
                xtile = const.tile([F, CHUNK_COLS], bf16, tag=f"xc{c}",
                                   name=f"xc_{c}")
                nc.sync.dma_start(
                    out=xtile,
                    in_=xt_ext[:, c * CHUNK_COLS:(c + 1) * CHUNK_COLS])
                xc.append(xtile)

            # transposed output accumulator (bf16)
            outT = const.tile([F, B_SHARD], bf16)

            for it in range(NTILES):
                xslice = xc[it // TILES_PER_CHUNK][
                    :, (it % TILES_PER_CHUNK) * P:(it % TILES_PER_CHUNK + 1) * P]

                # Q/K/V in batch-on-partition layout [128, (s,h,d)]
                psQ = psQKV.tile([P, F], f32, tag="qkv")
                psK = psQKV.tile([P, F], f32, tag="qkv")
                psV = psQKV.tile([P, F], f32, tag="qkv")
                nc.tensor.matmul(psQ, lhsT=xslice, rhs=w_sb["wq"][:],
                                 start=True, stop=True)
                nc.tensor.matmul(psK, lhsT=xslice, rhs=w_sb["wk"][:],
                                 start=True, stop=True)
                nc.tensor.matmul(psV, lhsT=xslice, rhs=w_sb["wv"][:],
                                 start=True, stop=True)

                q_sb = sbqkv.tile([P, S, H, HD], bf16, tag="q")
                k_sb = sbqkv.tile([P, S, H, HD], bf16, tag="k")
                v_sb = sbqkv.tile([P, S, H, HD], bf16, tag="v")
                for dst, ps, bn in ((q_sb, psQ, "bq"), (k_sb, psK, "bk"),
                                    (v_sb, psV, "bv")):
                    nc.vector.scalar_tensor_tensor(
                        out=dst[:].rearrange("p s h d -> p (s h d)"),
                        in0=ps[:], scalar=1.0, in1=b_sb[bn][:],
                        op0=mult, op1=add,
                    )

                # scores products + reduce over d
                qv = q_sb[:].transpose([0, 2, 1, 3])  # [p, h, q, d]
                kv = k_sb[:].transpose([0, 2, 1, 3])  # [p, h, k, d]
                prod = sbwork.tile([P, H, S, S, HD], bf16, tag="prod")
                scores = sbwork.tile([P, H, S, S], f32, tag="scores")
                for h in range(H):
                    nc.vector.tensor_mul(
                        prod[:, h],
                        qv[:, h].unsqueeze(2).broadcast_to([P, S, S, HD]),
                        kv[:, h].unsqueeze(1).broadcast_to([P, S, S, HD]),
                    )
                    nc.vector.tensor_reduce(
                        scores[:, h], prod[:, h], axis=mybir.AxisListType.X,
                        op=add,
                    )

                # softmax (scores bounded; no max subtraction needed)
                E = sbwork.tile([P, H, S, S], bf16, tag="E")
                nc.scalar.activation(E[:], scores[:],
                                     mybir.ActivationFunctionType.Exp)
                denom = sbwork.tile([P, H, S], f32, tag="denom")
                nc.vector.tensor_reduce(
                    denom[:], E[:], axis=mybir.AxisListType.X, op=add,
                )
                rden = sbwork.tile([P, H, S], f32, tag="rden")
                nc.vector.reciprocal(rden[:], denom[:])

                # ctx_unnorm = sum_k E[h,q,k] * V[h,k,d]
                vv = v_sb[:].transpose([0, 2, 3, 1])  # [p, h, d, k]
                prod2 = sbwork.tile([P, H, S, HD, S], bf16, tag="prod2")
                ctxu = sbwork.tile([P, H, S, HD], f32, tag="ctxu")
                for h in range(H):
                    nc.vector.tensor_mul(
                        prod2[:, h],
                        E[:, h].unsqueeze(2).broadcast_to([P, S, HD, S]),
                        vv[:, h].unsqueeze(1).broadcast_to([P, S, HD, S]),
                    )
                    nc.vector.tensor_reduce(
                        ctxu[:, h], prod2[:, h], axis=mybir.AxisListType.X,
                        op=add,
                    )

                # normalize; write in (s, h, d) order to match out-proj layout
                ctx_b = sbqkv.tile([P, S, H, HD], bf16, tag="ctx")
                nc.vector.tensor_mul(
                    ctx_b[:].transpose([0, 2, 1, 3]),
                    ctxu[:],
                    rden[:].unsqueeze(3).broadcast_to([P, H, S, HD]),
                )

                # out-proj (transposed): outT_tile = Wo_blk.T @ ctx^T + bo
                psC = psT.tile([F, P], bf16, tag="t")
                nc.tensor.transpose(
                    psC, ctx_b[:].rearrange("p s h d -> p (s h d)"), I128b[:]
                )
                ctxT = sbctxT.tile([F, P], bf16)
                nc.vector.tensor_copy(ctxT[:], psC[:])
                psOutT = psO.tile([F, P], f32)
                nc.tensor.matmul(psOutT, lhsT=w_sb["wo"][:], rhs=ctxT[:],
                                 start=True, stop=True)
                nc.vector.tensor_scalar_add(
                    outT[:, it * P:(it + 1) * P], psOutT[:], bo_sb[:],
                )

            # final output DMAs (few large transfers)
            for c in range(8):
                cols = B_SHARD // 8
                nc.sync.dma_start(
                    out=out_ext[:, c * cols:(c + 1) * cols],
                    in_=outT[:, c * cols:(c + 1) * cols])

    return nc


def get_graph():
    if "nc" not in _nc_cache:
        nc = _build_graph()
        nc.finalize()
        _nc_cache["nc"] = nc
    return _nc_cache["nc"]


def prepare_in_maps(x, Wq, bq, Wk, bk, Wv, bv, Wo, bo):
    bf16 = ml_dtypes.bfloat16
    sc = 1.0 / np.sqrt(np.float32(HD))
    seg = np.asarray(SEG)
    wq = _build_weight(Wq, sc).astype(bf16)
    wk = _build_weight(Wk).astype(bf16)
    wv = _build_weight(Wv).astype(bf16)
    wo_full = np.zeros((F, F), np.float32)
    for s in range(S):
        wo_full[s * D:(s + 1) * D, s * D:(s + 1) * D] = Wo.T
    wo = wo_full.astype(bf16)
    bqf = (bq[seg].reshape(F) * sc).astype(bf16)
    bkf = bk[seg].reshape(F).astype(bf16)
    bvf = bv[seg].reshape(F).astype(bf16)
    bof = np.tile(bo, S).astype(np.float32)
    ident = np.eye(P, dtype=bf16)

    xf = np.asarray(x, np.float32).reshape(B_TOTAL, F).astype(bf16)
    in_maps = []
    for c in range(N_CORES):
        shard = np.ascontiguousarray(
            xf[c * B_SHARD:(c + 1) * B_SHARD].T)  # [96, 32768]
        in_maps.append({"xt": shard, "wq": wq, "wk": wk, "wv": wv, "wo": wo,
                        "bq": bqf, "bk": bkf, "bv": bvf, "bo": bof,
                        "ident": ident})
    return in_maps


def kernel(x, Wq, bq, Wk, bk, Wv, bv, Wo, bo):
    nc = get_graph()
    in_maps = prepare_in_maps(x, Wq, bq, Wk, bk, Wv, bv, Wo, bo)
    res = run_bass_kernel_spmd(nc, in_maps, core_ids=list(range(N_CORES)))
    outs = [np.asarray(res.results[c]["out"]).astype(np.float32).T
            for c in range(N_CORES)]  # each [32768, 96]
    out = np.concatenate(outs, axis=0)
    return np.ascontiguousarray(out.reshape(B_TOTAL, S, D))


# revision 14
# speedup vs baseline: 1.0266x; 1.0266x over previous
"""Trainium2 Bass kernel for nn_Attention_17454747091547.

Segmented-projection 2-head attention over seq=16, head_dim=3, batch 262144.
Pure data parallel across 8 NeuronCores (32768 batch elements per core).

Host-side precompute folds the per-position segment weights into 96x96
block-diagonal projection matrices, so QKV + out-proj become single TensorE
matmuls. x arrives host-pre-transposed as [96, 32768] bf16 and is preloaded
into SBUF in chunks; output is accumulated transposed in SBUF (bf16) and
written out with a few large DMAs. This keeps every PE/DMA instruction at
<=1 sync-wait (walrus codegen limit): all PE operands except the x chunks
are produced by DVE, and a setup dummy matmul makes PE observe the DVE
semaphore before tile 0.

The attention core (scores/softmax/attn@V) runs on VectorE/ScalarE with
batch on partitions.
"""

import numpy as np
import ml_dtypes

import concourse.bass as bass
import concourse.tile as tile
from concourse import bacc
from concourse import mybir
from concourse.bass_utils import run_bass_kernel_spmd

SEG = [0, 1, 1, 1, 1, 1, 2, 2, 2, 3, 4, 4, 4, 4, 4, 4]
N_CORES = 8
B_TOTAL = 262144
B_SHARD = B_TOTAL // N_CORES  # 32768
P = 128
NTILES = B_SHARD // P  # 256
TILES_PER_CHUNK = 8
NCHUNKS = NTILES // TILES_PER_CHUNK  # 32
CHUNK_COLS = TILES_PER_CHUNK * P  # 1024
S = 16
D = 6
H = 2
HD = 3
F = S * D  # 96

_nc_cache = {}


def _build_weight(W, scale=1.0):
    """[5,6,6] -> [96,96] f32 blockdiag of W[seg[s]].T."""
    Wa = np.zeros((F, F), np.float32)
    for s in range(S):
        Wa[s * D:(s + 1) * D, s * D:(s + 1) * D] = W[SEG[s]].T * scale
    return Wa


def _build_graph():
    nc = bacc.Bacc()
    f32 = mybir.dt.float32
    bf16 = mybir.dt.bfloat16

    xt_ext = nc.declare_dram_parameter("xt", [F + 1, B_SHARD], bf16, isOutput=False)
    w_exts = {}
    b_exts = {}
    for nm in ["wq", "wk", "wv"]:
        w_exts[nm] = nc.declare_dram_parameter(nm, [F + 1, F], bf16, isOutput=False)
    w_exts["wo"] = nc.declare_dram_parameter("wo", [F, F], bf16, isOutput=False)
    bo_ext = nc.declare_dram_parameter("bo", [F], f32, isOutput=False)
    id_ext = nc.declare_dram_parameter("ident", [P, P], bf16, isOutput=False)
    out_ext = nc.declare_dram_parameter("out", [F, B_SHARD], bf16, isOutput=True)

    mult = mybir.AluOpType.mult
    add = mybir.AluOpType.add

    with tile.TileContext(nc) as tc:
        with (
            tc.tile_pool(name="const", bufs=1) as const,
            tc.tile_pool(name="sbqkv", bufs=2) as sbqkv,
            tc.tile_pool(name="sbwork", bufs=2) as sbwork,
            tc.tile_pool(name="sbctxT", bufs=2) as sbctxT,
            tc.tile_pool(name="psT", bufs=2, space="PSUM") as psT,
            tc.tile_pool(name="psQKV", bufs=4, space="PSUM") as psQKV,
            tc.tile_pool(name="psO", bufs=2, space="PSUM") as psO,
        ):
            # --- setup: stage every PE-read constant through DVE ---
            w_dma = {}
            w_sb = {}
            for nm in ["wq", "wk", "wv", "wo"]:
                rows = F + 1 if nm != "wo" else F
                w_dma[nm] = const.tile([rows, F], bf16, tag=nm + "d",
                                       name="wd_" + nm)
                nc.sync.dma_start(out=w_dma[nm], in_=w_exts[nm][:])
                w_sb[nm] = const.tile([rows, F], bf16, tag=nm, name="w_" + nm)
                nc.vector.tensor_copy(w_sb[nm][:], w_dma[nm][:])
            id_dma = const.tile([P, P], bf16)
            nc.sync.dma_start(out=id_dma, in_=id_ext[:])
            I128b = const.tile([P, P], bf16)
            nc.vector.tensor_copy(I128b[:], id_dma[:])
            bo_dma = const.tile([F, 1], f32)
            nc.sync.dma_start(out=bo_dma, in_=bo_ext[:].unsqueeze(1))
            bo_sb = const.tile([F, 1], f32)
            nc.vector.tensor_copy(bo_sb[:], bo_dma[:])

            # dummy matmul: PE observes the DVE setup tick, so tile-0 PE
            # instructions carry at most one sync-wait (walrus limit)
            psDummy = psT.tile([1, 1], f32, tag="t")
            nc.tensor.matmul(psDummy, lhsT=I128b[0:1, 0:1],
                             rhs=I128b[0:1, 0:1], start=True, stop=True)

            # x chunks: separate tiles so chunk DMAs have no mutual deps
            xc = []
            for c in range(NCHUNKS):
                xtile = const.tile([F + 1, CHUNK_COLS], bf16, tag=f"xc{c}",
                                   name=f"xc_{c}")
                nc.sync.dma_start(
                    out=xtile,
                    in_=xt_ext[:, c * CHUNK_COLS:(c + 1) * CHUNK_COLS])
                xc.append(xtile)

            # transposed output accumulator (bf16)
            outT = const.tile([F, B_SHARD], bf16)

            for it in range(NTILES):
                xslice = xc[it // TILES_PER_CHUNK][
                    :, (it % TILES_PER_CHUNK) * P:(it % TILES_PER_CHUNK + 1) * P]

                # Q/K/V in batch-on-partition layout [128, (s,h,d)]
                psQ = psQKV.tile([P, F], f32, tag="qkv")
                psK = psQKV.tile([P, F], f32, tag="qkv")
                psV = psQKV.tile([P, F], f32, tag="qkv")
                nc.tensor.matmul(psQ, lhsT=xslice, rhs=w_sb["wq"][:],
                                 start=True, stop=True)
                nc.tensor.matmul(psK, lhsT=xslice, rhs=w_sb["wk"][:],
                                 start=True, stop=True)
                nc.tensor.matmul(psV, lhsT=xslice, rhs=w_sb["wv"][:],
                                 start=True, stop=True)

                q_sb = sbqkv.tile([P, S, H, HD], bf16, tag="q")
                k_sb = sbqkv.tile([P, S, H, HD], bf16, tag="k")
                v_sb = sbqkv.tile([P, S, H, HD], bf16, tag="v")
                for dst, ps in ((q_sb, psQ), (k_sb, psK), (v_sb, psV)):
                    nc.scalar.activation(
                        dst[:].rearrange("p s h d -> p (s h d)"), ps[:],
                        mybir.ActivationFunctionType.Copy,
                    )

                # scores products + reduce over d
                qv = q_sb[:].transpose([0, 2, 1, 3])  # [p, h, q, d]
                kv = k_sb[:].transpose([0, 2, 1, 3])  # [p, h, k, d]
                prod = sbwork.tile([P, H, S, S, HD], bf16, tag="prod")
                scores = sbwork.tile([P, H, S, S], f32, tag="scores")
                for h in range(H):
                    nc.gpsimd.tensor_mul(
                        prod[:, h],
                        qv[:, h].unsqueeze(2).broadcast_to([P, S, S, HD]),
                        kv[:, h].unsqueeze(1).broadcast_to([P, S, S, HD]),
                    )
                    nc.vector.tensor_reduce(
                        scores[:, h], prod[:, h], axis=mybir.AxisListType.X,
                        op=add,
                    )

                # softmax (scores bounded; no max subtraction needed)
                E = sbwork.tile([P, H, S, S], bf16, tag="E")
                nc.scalar.activation(E[:], scores[:],
                                     mybir.ActivationFunctionType.Exp)
                denom = sbwork.tile([P, H, S], f32, tag="denom")
                nc.vector.tensor_reduce(
                    denom[:], E[:], axis=mybir.AxisListType.X, op=add,
                )
                rden = sbwork.tile([P, H, S], f32, tag="rden")
                nc.vector.reciprocal(rden[:], denom[:])

                # ctx_unnorm = sum_k E[h,q,k] * V[h,k,d]
                vv = v_sb[:].transpose([0, 2, 3, 1])  # [p, h, d, k]
                prod2 = sbwork.tile([P, H, S, HD, S], bf16, tag="prod2")
                ctxu = sbwork.tile([P, H, S, HD], f32, tag="ctxu")
                for h in range(H):
                    nc.gpsimd.tensor_mul(
                        prod2[:, h],
                        E[:, h].unsqueeze(2).broadcast_to([P, S, HD, S]),
                        vv[:, h].unsqueeze(1).broadcast_to([P, S, HD, S]),
                    )
                    nc.vector.tensor_reduce(
                        ctxu[:, h], prod2[:, h], axis=mybir.AxisListType.X,
                        op=add,
                    )

                # normalize; write in (s, h, d) order to match out-proj layout
                ctx_b = sbqkv.tile([P, S, H, HD], bf16, tag="ctx")
                nc.vector.tensor_mul(
                    ctx_b[:].transpose([0, 2, 1, 3]),
                    ctxu[:],
                    rden[:].unsqueeze(3).broadcast_to([P, H, S, HD]),
                )

                # out-proj (transposed): outT_tile = Wo_blk.T @ ctx^T + bo
                psC = psT.tile([F, P], bf16, tag="t")
                nc.tensor.transpose(
                    psC, ctx_b[:].rearrange("p s h d -> p (s h d)"), I128b[:]
                )
                ctxT = sbctxT.tile([F, P], bf16)
                nc.scalar.activation(ctxT[:], psC[:],
                                     mybir.ActivationFunctionType.Copy)
                psOutT = psO.tile([F, P], f32)
                nc.tensor.matmul(psOutT, lhsT=w_sb["wo"][:], rhs=ctxT[:],
                                 start=True, stop=True)
                nc.vector.tensor_scalar_add(
                    outT[:, it * P:(it + 1) * P], psOutT[:], bo_sb[:],
                )

            # final output DMAs (few large transfers)
            for c in range(8):
                cols = B_SHARD // 8
                nc.sync.dma_start(
                    out=out_ext[:, c * cols:(c + 1) * cols],
                    in_=outT[:, c * cols:(c + 1) * cols])

    return nc


def get_graph():
    if "nc" not in _nc_cache:
        nc = _build_graph()
        nc.finalize()
        _nc_cache["nc"] = nc
    return _nc_cache["nc"]


def _aug(Wblk, bvec):
    """[96,96] weight + [96] bias -> [97,96] with bias row."""
    return np.concatenate([Wblk, bvec[None, :]], axis=0)


def prepare_in_maps(x, Wq, bq, Wk, bk, Wv, bv, Wo, bo):
    bf16 = ml_dtypes.bfloat16
    sc = 1.0 / np.sqrt(np.float32(HD))
    seg = np.asarray(SEG)
    bqf = (bq[seg].reshape(F) * sc).astype(np.float32)
    bkf = bk[seg].reshape(F).astype(np.float32)
    bvf = bv[seg].reshape(F).astype(np.float32)
    wq = _aug(_build_weight(Wq, sc), bqf).astype(bf16)
    wk = _aug(_build_weight(Wk), bkf).astype(bf16)
    wv = _aug(_build_weight(Wv), bvf).astype(bf16)
    wo_full = np.zeros((F, F), np.float32)
    for s in range(S):
        wo_full[s * D:(s + 1) * D, s * D:(s + 1) * D] = Wo.T
    wo = wo_full.astype(bf16)
    bof = np.tile(bo, S).astype(np.float32)
    ident = np.eye(P, dtype=bf16)

    xf = np.asarray(x, np.float32).reshape(B_TOTAL, F).astype(bf16)
    ones = np.ones((1, B_SHARD), dtype=bf16)
    in_maps = []
    for c in range(N_CORES):
        shard = np.concatenate([np.ascontiguousarray(
            xf[c * B_SHARD:(c + 1) * B_SHARD].T), ones], axis=0)  # [97, B]
        in_maps.append({"xt": shard, "wq": wq, "wk": wk, "wv": wv, "wo": wo,
                        "bo": bof, "ident": ident})
    return in_maps


def kernel(x, Wq, bq, Wk, bk, Wv, bv, Wo, bo):
    nc = get_graph()
    in_maps = prepare_in_maps(x, Wq, bq, Wk, bk, Wv, bv, Wo, bo)
    res = run_bass_kernel_spmd(nc, in_maps, core_ids=list(range(N_CORES)))
    outs = [np.asarray(res.results[c]["out"]).astype(np.float32).T
            for c in range(N_CORES)]  # each [32768, 96]
    out = np.concatenate(outs, axis=0)
    return np.ascontiguousarray(out.reshape(B_TOTAL, S, D))


# revision 15
# speedup vs baseline: 1.2742x; 1.2411x over previous
"""Trainium2 Bass kernel for nn_Attention_17454747091547.

Segmented-projection 2-head attention over seq=16, head_dim=3, batch 262144.
Pure data parallel across 8 NeuronCores (32768 batch elements per core).

Host-side precompute folds the per-position segment weights into 96x96
block-diagonal projection matrices, so QKV + out-proj become single TensorE
matmuls. x arrives host-pre-transposed as [96, 32768] bf16 and is preloaded
into SBUF in chunks; output is accumulated transposed in SBUF (bf16) and
written out with a few large DMAs. This keeps every PE/DMA instruction at
<=1 sync-wait (walrus codegen limit): all PE operands except the x chunks
are produced by DVE, and a setup dummy matmul makes PE observe the DVE
semaphore before tile 0.

The attention core (scores/softmax/attn@V) runs on VectorE/ScalarE with
batch on partitions.
"""

import numpy as np
import ml_dtypes

import concourse.bass as bass
import concourse.tile as tile
from concourse import bacc
from concourse import mybir
from concourse.bass_utils import run_bass_kernel_spmd

SEG = [0, 1, 1, 1, 1, 1, 2, 2, 2, 3, 4, 4, 4, 4, 4, 4]
N_CORES = 8
B_TOTAL = 262144
B_SHARD = B_TOTAL // N_CORES  # 32768
P = 128
NTILES = B_SHARD // P  # 256
TILES_PER_CHUNK = 8
NCHUNKS = NTILES // TILES_PER_CHUNK  # 32
CHUNK_COLS = TILES_PER_CHUNK * P  # 1024
S = 16
D = 6
H = 2
HD = 3
F = S * D  # 96

_nc_cache = {}


def _build_weight(W, scale=1.0):
    """[5,6,6] -> [96,96] f32 blockdiag of W[seg[s]].T."""
    Wa = np.zeros((F, F), np.float32)
    for s in range(S):
        Wa[s * D:(s + 1) * D, s * D:(s + 1) * D] = W[SEG[s]].T * scale
    return Wa


def _build_graph():
    nc = bacc.Bacc()
    f32 = mybir.dt.float32
    bf16 = mybir.dt.bfloat16

    xt_ext = nc.declare_dram_parameter("xt", [F + 1, B_SHARD], bf16, isOutput=False)
    w_exts = {}
    b_exts = {}
    for nm in ["wq", "wk", "wv"]:
        w_exts[nm] = nc.declare_dram_parameter(nm, [F + 1, F], bf16, isOutput=False)
    w_exts["wo"] = nc.declare_dram_parameter("wo", [F, F], bf16, isOutput=False)
    bo_ext = nc.declare_dram_parameter("bo", [F], f32, isOutput=False)
    id_ext = nc.declare_dram_parameter("ident", [P, P], bf16, isOutput=False)
    out_ext = nc.declare_dram_parameter("out", [F, B_SHARD], bf16, isOutput=True)

    mult = mybir.AluOpType.mult
    add = mybir.AluOpType.add

    with tile.TileContext(nc) as tc:
        with (
            tc.tile_pool(name="const", bufs=1) as const,
            tc.tile_pool(name="sbqkv", bufs=3) as sbqkv,
            tc.tile_pool(name="sbwork", bufs=3) as sbwork,
            tc.tile_pool(name="sbctxT", bufs=3) as sbctxT,
            tc.tile_pool(name="psT", bufs=2, space="PSUM") as psT,
            tc.tile_pool(name="psQKV", bufs=4, space="PSUM") as psQKV,
            tc.tile_pool(name="psO", bufs=2, space="PSUM") as psO,
        ):
            # --- setup: stage every PE-read constant through DVE ---
            w_dma = {}
            w_sb = {}
            for nm in ["wq", "wk", "wv", "wo"]:
                rows = F + 1 if nm != "wo" else F
                w_dma[nm] = const.tile([rows, F], bf16, tag=nm + "d",
                                       name="wd_" + nm)
                nc.sync.dma_start(out=w_dma[nm], in_=w_exts[nm][:])
                w_sb[nm] = const.tile([rows, F], bf16, tag=nm, name="w_" + nm)
                nc.vector.tensor_copy(w_sb[nm][:], w_dma[nm][:])
            id_dma = const.tile([P, P], bf16)
            nc.sync.dma_start(out=id_dma, in_=id_ext[:])
            I128b = const.tile([P, P], bf16)
            nc.vector.tensor_copy(I128b[:], id_dma[:])
            bo_dma = const.tile([F, 1], f32)
            nc.sync.dma_start(out=bo_dma, in_=bo_ext[:].unsqueeze(1))
            bo_sb = const.tile([F, 1], f32)
            nc.vector.tensor_copy(bo_sb[:], bo_dma[:])

            # dummy matmul: PE observes the DVE setup tick, so tile-0 PE
            # instructions carry at most one sync-wait (walrus limit)
            psDummy = psT.tile([1, 1], f32, tag="t")
            nc.tensor.matmul(psDummy, lhsT=I128b[0:1, 0:1],
                             rhs=I128b[0:1, 0:1], start=True, stop=True)

            # x chunks: separate tiles so chunk DMAs have no mutual deps
            xc = []
            for c in range(NCHUNKS):
                xtile = const.tile([F + 1, CHUNK_COLS], bf16, tag=f"xc{c}",
                                   name=f"xc_{c}")
                nc.sync.dma_start(
                    out=xtile,
                    in_=xt_ext[:, c * CHUNK_COLS:(c + 1) * CHUNK_COLS])
                xc.append(xtile)

            # transposed output accumulator (bf16)
            outT = const.tile([F, B_SHARD], bf16)

            for it in range(NTILES):
                xslice = xc[it // TILES_PER_CHUNK][
                    :, (it % TILES_PER_CHUNK) * P:(it % TILES_PER_CHUNK + 1) * P]

                # Q/K/V in batch-on-partition layout [128, (s,h,d)]
                psQ = psQKV.tile([P, F], f32, tag="qkv")
                psK = psQKV.tile([P, F], f32, tag="qkv")
                psV = psQKV.tile([P, F], f32, tag="qkv")
                nc.tensor.matmul(psQ, lhsT=xslice, rhs=w_sb["wq"][:],
                                 start=True, stop=True)
                nc.tensor.matmul(psK, lhsT=xslice, rhs=w_sb["wk"][:],
                                 start=True, stop=True)
                nc.tensor.matmul(psV, lhsT=xslice, rhs=w_sb["wv"][:],
                                 start=True, stop=True)

                q_sb = sbqkv.tile([P, S, H, HD], bf16, tag="q")
                k_sb = sbqkv.tile([P, S, H, HD], bf16, tag="k")
                v_sb = sbqkv.tile([P, H, HD, S], bf16, tag="v")
                for dst, ps in ((q_sb, psQ), (k_sb, psK), (v_sb, psV)):
                    nc.scalar.activation(
                        dst[:].rearrange("p a b c -> p (a b c)"), ps[:],
                        mybir.ActivationFunctionType.Copy,
                    )

                # scores products + reduce over d
                qv = q_sb[:].transpose([0, 2, 1, 3])  # [p, h, q, d]
                kv = k_sb[:].transpose([0, 2, 1, 3])  # [p, h, k, d]
                prod = sbwork.tile([P, H, S, S, HD], bf16, tag="prod")
                scores = sbwork.tile([P, H, S, S], f32, tag="scores")
                for h in range(H):
                    nc.gpsimd.tensor_mul(
                        prod[:, h],
                        qv[:, h].unsqueeze(2).broadcast_to([P, S, S, HD]),
                        kv[:, h].unsqueeze(1).broadcast_to([P, S, S, HD]),
                    )
                    nc.vector.tensor_reduce(
                        scores[:, h], prod[:, h], axis=mybir.AxisListType.X,
                        op=add,
                    )

                # softmax (scores bounded; no max subtraction needed)
                E = sbwork.tile([P, H, S, S], bf16, tag="E")
                nc.scalar.activation(E[:], scores[:],
                                     mybir.ActivationFunctionType.Exp)
                denom = sbwork.tile([P, H, S], f32, tag="denom")
                nc.vector.tensor_reduce(
                    denom[:], E[:], axis=mybir.AxisListType.X, op=add,
                )
                rden = sbwork.tile([P, H, S], f32, tag="rden")
                nc.vector.reciprocal(rden[:], denom[:])

                # ctx_unnorm[h,q,d] = sum_k E[h,q,k] * V[h,d,k]
                # layout keeps innermost k stride-1 on all APs -> DVE 2x mode
                prod2 = sbwork.tile([P, H, S, HD, S], bf16, tag="prod2")
                ctxu = sbwork.tile([P, H, S, HD], f32, tag="ctxu")
                for h in range(H):
                    nc.vector.tensor_mul(
                        prod2[:, h],
                        E[:, h].unsqueeze(2).broadcast_to([P, S, HD, S]),
                        v_sb[:, h].unsqueeze(1).broadcast_to([P, S, HD, S]),
                    )
                    nc.vector.tensor_reduce(
                        ctxu[:, h], prod2[:, h], axis=mybir.AxisListType.X,
                        op=add,
                    )

                # normalize; write in (s, h, d) order to match out-proj layout
                ctx_b = sbqkv.tile([P, S, H, HD], bf16, tag="ctx")
                nc.vector.tensor_mul(
                    ctx_b[:].transpose([0, 2, 1, 3]),
                    ctxu[:],
                    rden[:].unsqueeze(3).broadcast_to([P, H, S, HD]),
                )

                # out-proj (transposed): outT_tile = Wo_blk.T @ ctx^T + bo
                psC = psT.tile([F, P], bf16, tag="t")
                nc.tensor.transpose(
                    psC, ctx_b[:].rearrange("p s h d -> p (s h d)"), I128b[:]
                )
                ctxT = sbctxT.tile([F, P], bf16)
                nc.scalar.activation(ctxT[:], psC[:],
                                     mybir.ActivationFunctionType.Copy)
                psOutT = psO.tile([F, P], f32)
                nc.tensor.matmul(psOutT, lhsT=w_sb["wo"][:], rhs=ctxT[:],
                                 start=True, stop=True)
                nc.vector.tensor_scalar_add(
                    outT[:, it * P:(it + 1) * P], psOutT[:], bo_sb[:],
                )

            # final output DMAs (few large transfers)
            for c in range(8):
                cols = B_SHARD // 8
                nc.sync.dma_start(
                    out=out_ext[:, c * cols:(c + 1) * cols],
                    in_=outT[:, c * cols:(c + 1) * cols])

    return nc


def get_graph():
    if "nc" not in _nc_cache:
        nc = _build_graph()
        nc.finalize()
        _nc_cache["nc"] = nc
    return _nc_cache["nc"]


def _aug(Wblk, bvec):
    """[96,96] weight + [96] bias -> [97,96] with bias row."""
    return np.concatenate([Wblk, bvec[None, :]], axis=0)


def prepare_in_maps(x, Wq, bq, Wk, bk, Wv, bv, Wo, bo):
    bf16 = ml_dtypes.bfloat16
    sc = 1.0 / np.sqrt(np.float32(HD))
    seg = np.asarray(SEG)
    bqf = (bq[seg].reshape(F) * sc).astype(np.float32)
    bkf = bk[seg].reshape(F).astype(np.float32)
    bvf = bv[seg].reshape(F).astype(np.float32)
    wq = _aug(_build_weight(Wq, sc), bqf).astype(bf16)
    wk = _aug(_build_weight(Wk), bkf).astype(bf16)
    # V projection columns permuted to (h, d, k) order
    perm = np.empty(F, np.int64)
    for h in range(H):
        for d in range(HD):
            for k in range(S):
                perm[h * HD * S + d * S + k] = k * D + h * HD + d
    wv = _aug(_build_weight(Wv), bvf)[:, perm].astype(bf16)
    wo_full = np.zeros((F, F), np.float32)
    for s in range(S):
        wo_full[s * D:(s + 1) * D, s * D:(s + 1) * D] = Wo.T
    wo = wo_full.astype(bf16)
    bof = np.tile(bo, S).astype(np.float32)
    ident = np.eye(P, dtype=bf16)

    xf = np.asarray(x, np.float32).reshape(B_TOTAL, F).astype(bf16)
    ones = np.ones((1, B_SHARD), dtype=bf16)
    in_maps = []
    for c in range(N_CORES):
        shard = np.concatenate([np.ascontiguousarray(
            xf[c * B_SHARD:(c + 1) * B_SHARD].T), ones], axis=0)  # [97, B]
        in_maps.append({"xt": shard, "wq": wq, "wk": wk, "wv": wv, "wo": wo,
                        "bo": bof, "ident": ident})
    return in_maps


def kernel(x, Wq, bq, Wk, bk, Wv, bv, Wo, bo):
    nc = get_graph()
    in_maps = prepare_in_maps(x, Wq, bq, Wk, bk, Wv, bv, Wo, bo)
    res = run_bass_kernel_spmd(nc, in_maps, core_ids=list(range(N_CORES)))
    outs = [np.asarray(res.results[c]["out"]).astype(np.float32).T
            for c in range(N_CORES)]  # each [32768, 96]
    out = np.concatenate(outs, axis=0)
    return np.ascontiguousarray(out.reshape(B_TOTAL, S, D))
